# revision 28
# baseline (speedup 1.0000x reference)
"""4-bit comparator (a>b, a==b) over [8388608, 4] binary spike inputs.

Strategy: rows are data-parallel across 8 NeuronCores. On each core the
integer values of the 4-bit operands are compared via their weighted
difference d = sum_j w_j*(A_j - B_j), w = [8,4,2,1] (MSB first), computed
on the TensorEngine as accumulating matmuls with scaled-identity
stationary weights. Inputs are {0,1} so fp8(e4m3) holds them exactly:
the host casts f32 -> fp8 and packs each core's slice into ONE tensor,
chunked bit-planar ([A planes | B planes] per 512-row chunk) so PE
moving access patterns are contiguous and plane pairs form DoubleRow
k-subtiles (fp8 perf mode: 2 MACs/cell/cycle, 4 matmuls per group).

Pipelining: 16 chunk buffers are preallocated and each chunk DMA gets
its own single-use semaphore, so both HWDGE rings (sync + act) prefetch
their whole half of the stream with zero issuer waits. 512 KiB chunks
keep PE idle gaps far below the ~3.4us HAM window (PE clock stays at
2.4 GHz) and dummy warm-up matmuls ramp the clock before chunk 0 lands.

Outputs are PAIR-PACKED: odd chunks use x8 weights, so psum_odd = 8*d;
Scalar emits s_e = sign(d_even) in {-1,0,1}, Vector emits s_o8 =
clamp(8*d_odd, +-8) = 8*sign(d_odd) and adds them into one fp8 byte
per row-pair (19 distinct exact values), decoded on host by LUT.
HBM per core: 8 MiB in + 0.5 MiB out.
"""

import os
import sys

if "/opt/trn_rl_repo" not in sys.path:
    sys.path.insert(0, "/opt/trn_rl_repo")

import numpy as np
import ml_dtypes

N_ROWS = 8_388_608
N_CORES = 8
R = N_ROWS // N_CORES          # rows per core = 1,048,576
P = 128                        # SBUF partitions
MPP = R // P                   # rows per partition = 8192
NCH = 16                       # chunks per core (= psum groups)
TCH = MPP // NCH               # rows per partition per chunk = 512
CHE = 8 * TCH                  # AB elems per partition per chunk = 4096
NPR = NCH // 2                 # output pairs = 8
NPS = 8                        # psum banks in flight
W_BITS = (8.0, 4.0, 2.0, 1.0)  # MSB-first bit weights

DR = os.environ.get("DR", "1") == "1"   # fp8 DoubleRow perf mode
WARM = int(os.environ.get("WARM", "28"))  # PE warm-up dummy matmuls

# input DMA schedule as half-open group ranges. Early chunk-pairs are
# fused into 1 MiB DMAs (8 KiB partition lines move ~20% faster than
# 4 KiB), singles at the end keep the tail short. Weights head the sync
# ring; the act ring has been observed to start ~0.5-1.8us later.
SYNC_DMAS = ((0, 2), (4, 6), (8, 10), (12, 13), (14, 15))
ACT_DMAS = ((2, 4), (6, 8), (10, 12), (13, 14), (15, 16))
# output slabs (pair ranges): small ones at the end shrink the tail
SYNC_SLABS = ((0, 3), (6, 7))
ACT_SLABS = ((3, 6), (7, 8))

_CACHE = {}


def _build(mpp=MPP, dr=DR):
    import concourse.bass as bass
    import concourse.mybir as mybir

    nch = NCH
    tch = mpp // nch
    che = 8 * tch
    npr = nch // 2
    AluOp = mybir.AluOpType

    nc = bass.Bass(trn_type="TRN2")
    f8 = mybir.dt.float8e4
    f32 = mybir.dt.float32
    AB = nc.dram_tensor("AB", [P, nch * che], f8, kind="ExternalInput")
    out = nc.dram_tensor("out", [P, mpp // 2], f8, kind="ExternalOutput")

    # stationary weights: rows 0..3 = +w_k*I (A planes), 4..7 = -w_k*I.
    # Odd chunks get psum = 8*d via host-scaled inputs ({0,8} not {0,1}).
    wnp = np.zeros((P, 8, P), dtype=ml_dtypes.float8_e4m3)
    for k in range(4):
        for p in range(P):
            wnp[p, k, p] = W_BITS[k]
            wnp[p, 4 + k, p] = -W_BITS[k]
    wdram = nc.inline_tensor(wnp, name="wconst")

    from contextlib import ExitStack
    with ExitStack() as ctx:
        ec = ctx.enter_context
        wt = ec(nc.sbuf_tensor("wt", [P, 8, P], f8))
        cbig = ec(nc.sbuf_tensor("cbig", [P, nch * che], f8))
        te = [ec(nc.sbuf_tensor(f"te{i}", [P, tch], f8)) for i in range(2)]
        to = [ec(nc.sbuf_tensor(f"to{i}", [P, tch], f8)) for i in range(2)]
        ot = ec(nc.sbuf_tensor("ot", [P, npr * tch], f8))
        pss = [ec(nc.psum_tensor(f"ps{i}", [P, tch], f32)) for i in range(NPS)]
        scr = ec(nc.sbuf_tensor("scr", [P, 1], f8))
        s_w = ec(nc.semaphore(name="s_w"))
        ndma = len(SYNC_DMAS) + len(ACT_DMAS)
        s_in = [ec(nc.semaphore(name=f"s_in{i}")) for i in range(ndma)]
        # group -> input-DMA index
        gmap = {}
        for i, (a, b) in enumerate(SYNC_DMAS + ACT_DMAS):
            for g in range(a, b):
                gmap[g] = i
        s_peg = ec(nc.semaphore(name="s_peg"))
        s_cmpE = ec(nc.semaphore(name="s_cmpE"))
        s_cmpO = ec(nc.semaphore(name="s_cmpO"))
        s_add = ec(nc.semaphore(name="s_add"))
        s_out = ec(nc.semaphore(name="s_out"))
        block = ec(nc.Block())

        nslab = len(SYNC_SLABS) + len(ACT_SLABS)

        @block.sync
        def _(sync):
            sync.dma_start(wt[:], wdram[:]).then_inc(s_w, 16)
            for i, (a, b) in enumerate(SYNC_DMAS):
                sl = slice(a * che, b * che)
                sync.dma_start(cbig[:, sl], AB[:, sl]).then_inc(s_in[i], 16)
            for lo, hi in SYNC_SLABS:
                sync.wait_ge(s_add, hi)
                slab = slice(lo * tch, hi * tch)
                sync.dma_start(out[:, slab], ot[:, slab]).then_inc(s_out, 16)
            sync.wait_ge(s_out, 16 * nslab)

        @block.scalar
        def _(act):
            # act ring: its chunk half, then a dummy 1-col sign to preload
            # the activation table before the first real one
            for i, (a, b) in enumerate(ACT_DMAS):
                sl = slice(a * che, b * che)
                act.dma_start(cbig[:, sl], AB[:, sl]).then_inc(
                    s_in[len(SYNC_DMAS) + i], 16)
            act.wait_ge(s_w, 16)
            nc.scalar.sign(scr[:], wt[:, 0, 0:1])
            slabs = list(ACT_SLABS)
            for m in range(npr):
                if m >= 2:
                    # te[m%2] reused from pair m-2: its add is done
                    act.wait_ge(s_add, m - 1)
                act.wait_ge(s_peg, 2 * m + 1)
                nc.scalar.sign(
                    te[m % 2][:], pss[(2 * m) % NPS][:]
                ).then_inc(s_cmpE, 1)
                while slabs and slabs[0][1] <= m + 1:
                    lo, hi = slabs.pop(0)
                    act.wait_ge(s_add, hi)
                    slab = slice(lo * tch, hi * tch)
                    act.dma_start(out[:, slab], ot[:, slab]).then_inc(s_out, 16)
            for lo, hi in slabs:
                act.wait_ge(s_add, hi)
                slab = slice(lo * tch, hi * tch)
                act.dma_start(out[:, slab], ot[:, slab]).then_inc(s_out, 16)

        @block.vector
        def _(dve):
            for m in range(npr):
                dve.wait_ge(s_peg, 2 * m + 2)
                nc.vector.tensor_scalar(
                    out=to[m % 2][:], in0=pss[(2 * m + 1) % NPS][:],
                    scalar1=8.0, scalar2=-8.0,
                    op0=AluOp.min, op1=AluOp.max,
                ).then_inc(s_cmpO, 1)
                dve.wait_ge(s_cmpE, m + 1)
                dve.wait_ge(s_cmpO, m + 1)  # trivially satisfied, for the RD
                nc.vector.tensor_tensor(
                    out=ot[:, m * tch:(m + 1) * tch],
                    in0=te[m % 2][:], in1=to[m % 2][:], op=AluOp.add,
                ).then_inc(s_add, 1)

        @block.tensor
        def _(pe):
            pe.wait_ge(s_w, 16)
            # warm the HAM clock gate while chunk 0 is still in flight
            wfd = min(tch, P)
            for _ in range(WARM):
                nc.tensor.matmul(
                    pss[NPS - 1][:, 0:wfd],
                    wt[:, 0:2, :],
                    wt[:, 0:2, 0:wfd],
                    start=True, stop=True,
                    perf_mode=mybir.MatmulPerfMode.DoubleRow,
                    skip_group_check=True,
                )
            for g in range(nch):
                pe.wait_ge(s_in[gmap[g]], 16)
                if g >= NPS:
                    # psum slot g%NPS reused from group g-NPS: consumed
                    if g % 2 == 0:
                        pe.wait_ge(s_cmpE, (g - NPS) // 2 + 1)
                    else:
                        pe.wait_ge(s_cmpO, (g - NPS) // 2 + 1)
                cv = cbig[:, g * che:(g + 1) * che].rearrange(
                    "p (j t) -> p j t", j=8)
                if dr:
                    for idx in range(4):
                        mm = nc.tensor.matmul(
                            pss[g % NPS][:],
                            wt[:, 2 * idx:2 * idx + 2, :],
                            cv[:, 2 * idx:2 * idx + 2, :],
                            start=(idx == 0),
                            stop=(idx == 3),
                            perf_mode=mybir.MatmulPerfMode.DoubleRow,
                            skip_group_check=(g % NPS == NPS - 1),
                        )
                else:
                    for ki in range(8):
                        mm = nc.tensor.matmul(
                            pss[g % NPS][:],
                            wt[:, ki, :],
                            cv[:, ki, :],
                            start=(ki == 0),
                            stop=(ki == 7),
                            skip_group_check=(g % NPS == NPS - 1),
                        )
                mm.then_inc(s_peg, 1)

    return nc


def _get_nc():
    if "nc" not in _CACHE:
        _CACHE["nc"] = _build()
    return _CACHE["nc"]


FP8_ONE = np.uint8(0x38)  # e4m3 +1.0

# byte LUTs for the packed output o = s_e + 8*s_o, s in {-1,0,1}
_LUT_GT_E = np.zeros(256, dtype=np.float32)
_LUT_EQ_E = np.zeros(256, dtype=np.float32)
_LUT_GT_O = np.zeros(256, dtype=np.float32)
_LUT_EQ_O = np.zeros(256, dtype=np.float32)
for _se in (-1, 0, 1):
    for _so in (-1, 0, 1):
        _b = int(np.float32(_se + 8 * _so).astype(ml_dtypes.float8_e4m3)
                 .view(np.uint8))
        _LUT_GT_E[_b] = float(_se > 0)
        _LUT_EQ_E[_b] = float(_se == 0)
        _LUT_GT_O[_b] = float(_so > 0)
        _LUT_EQ_O[_b] = float(_so == 0)


FP8_EIGHT = np.uint8(0x50)  # e4m3 +8.0


def _pack_ab(A, B, sl, mpp=MPP):
    """Core slices [R,4] f32 {0,1} -> [P, 2*4*mpp] fp8, chunk layout:
    free = c*8*TCH + ab*4*TCH + k*TCH + t for row r = p*mpp + c*TCH + t.
    Odd chunks carry {0, 8.0} instead of {0, 1.0} so psum_odd = 8*d with
    a single set of stationary weights.
    """
    tch = mpp // NCH

    def planar(x):
        # [P, NCH, TCH, 4] -> [P, NCH, 4, TCH]
        return (np.asarray(x[sl]).reshape(P, NCH, tch, 4) != 0).transpose(0, 1, 3, 2)

    ab = np.stack([planar(A), planar(B)], axis=2)  # [P, NCH, 2, 4, TCH]
    one = np.empty((1, NCH, 1, 1, 1), dtype=np.uint8)
    one[:, 0::2] = FP8_ONE
    one[:, 1::2] = FP8_EIGHT
    ab = ab.astype(np.uint8) * one
    return np.ascontiguousarray(ab).reshape(P, 8 * mpp).view(ml_dtypes.float8_e4m3)


def _decode(o_bytes, mpp=MPP):
    """Packed out [P, mpp//2] bytes -> (gt, eq) flat [P*mpp] f32."""
    tch = mpp // NCH
    o3 = o_bytes.reshape(P, NCH // 2, tch)
    gt = np.empty((P, NCH, tch), dtype=np.float32)
    eq = np.empty((P, NCH, tch), dtype=np.float32)
    gt[:, 0::2] = _LUT_GT_E[o3]
    gt[:, 1::2] = _LUT_GT_O[o3]
    eq[:, 0::2] = _LUT_EQ_E[o3]
    eq[:, 1::2] = _LUT_EQ_O[o3]
    return gt.reshape(-1), eq.reshape(-1)


def kernel(A, B, trace=False):
    from concourse import bass_utils

    A = np.asarray(A)
    B = np.asarray(B)
    assert A.shape == (N_ROWS, 4) and B.shape == (N_ROWS, 4), (A.shape, B.shape)

    in_maps = []
    for i in range(N_CORES):
        sl = slice(i * R, (i + 1) * R)
        in_maps.append({"AB": _pack_ab(A, B, sl)})

    nc = _get_nc()
    res = bass_utils.run_bass_kernel_spmd(
        nc, in_maps, core_ids=list(range(N_CORES)), trace=trace,
    )
    _CACHE["last_results"] = res

    gt = np.empty((N_ROWS,), dtype=np.float32)
    eq = np.empty((N_ROWS,), dtype=np.float32)
    for i in range(N_CORES):
        o = np.asarray(res.results[i]["out"]).view(np.uint8).reshape(-1)
        sl = slice(i * R, (i + 1) * R)
        gt[sl], eq[sl] = _decode(o)
    return gt.reshape(N_ROWS, 1), eq.reshape(N_ROWS, 1)
